# revision 12
# baseline (speedup 1.0000x reference)
"""DTW frames layer on 8 Trainium2 NeuronCores.

Reference computation (per (n, k) problem):
    cost[p, w] = max(0, ||x[n, :, w] - patts[k, :, p]||^2)          (P=32, W=128)
    dtw[0, w]  = cumsum_w cost[0, w]
    dtw[p, 0]  = cumsum_p cost[p, 0]
    dtw[p, w]  = cost[p, w] + min(dtw[p, w-1], dtw[p-1, w-1], dtw[p-1, w])
    out        = sqrt(dtw[:, -32:]) / 32

Strategy (v2):
  - Data-parallel over batch n: each of the 8 cores owns n_loc = 8 rows of x,
    patterns replicated. Per core, two problem tiles of 128 partitions each
    (4 n x 32 k).
  - The cost matrix is produced DIRECTLY in scan layout (partitions =
    (nn, k), free = (p, w)) by delta-masked bf16 matmuls, eliminating the
    4 MB SBUF->SBUF partition regroup of v1 entirely:
      PSUM[(nn,k), (i,w)] = sum_{d',n',p'} AA[(d',n',p'), (nn,k)]
                                         * BB[(d',n',p'), (i,w)]
    with AA = delta(nn==n') * L[d',k,4pc+p'] and BB = delta(i==p') *
    R[d',4t+n',w], where L/R carry [-2*patts | pnorm | 1] and
    [x | 1 | xnorm]. Contraction = (d' in 10, n' in 4, p' in 4) = 160 rows,
    split into a 128-row and a 32-row matmul accumulating into one PSUM
    bank. Operands are host-prepared (masking is free on the CPU); bf16
    keeps the PE ~5x faster than fp32 and costs only ~8e-4 relative error.
  - ReLU on eviction (ACT) drops each PSUM bank into C[t] at the final
    (p, w) column layout - pure dense 2D ops, no data movement.
  - DTW row recurrence on the DVE tensor_tensor_scan instruction:
    state = (m[t] min state) add c[t], one instruction per table row, where
    m[t] = min(dtw[p-1, t-1], dtw[p-1, t]) is one shifted tensor_tensor min.
    Column 0 of the m operand stays at +BIG so element 0 of each scan is
    initial + cost (the first-column cumsum), initial = prev row's column 0.
    The two tiles' chains are emitted row-interleaved so the in-order DVE
    fills one chain's dependency-turnaround gap with the other chain's op.
  - Output is pipelined per row-quad: after rows 4q..4q+3 of a tile are
    scanned, one Sqrt activation (scale=1/1024 => sqrt(dtw)/32) and one
    contiguous 64 KB DMA ship them to HBM, so only the last quad's store
    lands in the kernel tail.
"""

import numpy as np
import ml_dtypes

import concourse.bass as bass
import concourse.mybir as mybir
import concourse.tile as tile
from concourse.bass_utils import run_bass_kernel_spmd

N, D, W = 64, 8, 128      # x: (N, D, W)
K, P = 32, 32             # patts: (K, D, P)
WO = 32                   # output keeps last WO columns of the DTW table
NCORES = 8
NLOC = N // NCORES        # 8 batch rows per core
NT = 2                    # problem tiles per core: (4 n x 32 k) = 128 partitions
NCH = P // 4              # p-quad chunks per tile
DAUG = D + 2              # augmented d' rows: [-2*patts | pnorm | 1]
BIG = 1e30

f32 = mybir.dt.float32
bf16 = mybir.dt.bfloat16

# Engine for the shifted-min (m) op: "vector" or "gpsimd".
TT_ENGINE = "vector"


def build_program() -> bass.Bass:
    from concourse.tile import add_dep_helper

    nc = bass.Bass()
    # hi: [128, 2048] = AA_hi (8 chunks x 128) | BB_hi (2 tiles x 512)
    # lo: [32, 2048]  = AA_lo | BB_lo  (the d' in {8, 9} norm rows)
    hi_d = nc.dram_tensor("hi", (128, 2048), bf16, kind="ExternalInput")
    lo_d = nc.dram_tensor("lo", (32, 2048), bf16, kind="ExternalInput")
    out_d = nc.dram_tensor("out", (NLOC, K, P, WO), f32, kind="ExternalOutput")

    with tile.TileContext(nc) as tc:
        with (
            tc.tile_pool(name="consts", bufs=1) as consts,
            tc.tile_pool(name="psum", bufs=8, space="PSUM") as psum_pool,
            tc.tile_pool(name="cbuf", bufs=1) as c_pool,
            tc.tile_pool(name="dbuf", bufs=1) as d_pool,
            tc.tile_pool(name="mbuf", bufs=1) as m_pool,
            tc.tile_pool(name="obuf", bufs=1) as o_pool,
        ):
            hi_s = consts.tile([128, 2048], bf16)
            lo_s = consts.tile([32, 2048], bf16)
            nc.sync.dma_start(out=hi_s, in_=hi_d[:, :])
            nc.sync.dma_start(out=lo_s, in_=lo_d[:, :])
            AAh = hi_s[:, 0:NCH * 128]
            BBh = hi_s[:, NCH * 128:NCH * 128 + NT * 512]
            AAl = lo_s[:, 0:NCH * 128]
            BBl = lo_s[:, NCH * 128:NCH * 128 + NT * 512]

            C = [
                c_pool.tile([128, P * W], f32, tag=f"C{t}", name=f"C{t}")
                for t in range(NT)
            ]
            Dt = [
                d_pool.tile([128, P * W], f32, tag=f"D{t}", name=f"D{t}")
                for t in range(NT)
            ]
            mts = [
                m_pool.tile([128, W], f32, tag=f"mt{t}", name=f"mt{t}")
                for t in range(NT)
            ]
            bigm = m_pool.tile([128, W], f32, tag="bigm", name="bigm")
            ots = [
                o_pool.tile([128, P, WO], f32, tag=f"ot{t}", name=f"ot{t}")
                for t in range(NT)
            ]
            # DVE wait absorbers: the scan ISA op (S2S2D2_STT) has no sync
            # wait slot, so a tiny same-engine tensor_copy soaks each
            # quad's cross-engine evict semaphore ahead of its first scan.
            factd = [
                consts.tile([1, 1], f32, name=f"factd{i}", tag=f"factd{i}")
                for i in range(NT * NCH)
            ]
            facta = consts.tile([1, 1], f32, name="facta", tag="facta")
            nc.vector.memset(bigm, BIG)
            for t in range(NT):
                nc.vector.memset(mts[t], BIG)

            # Cost production: one PSUM bank per (pc, t) in scan layout.
            last_mm = None
            for pc in range(NCH):
                for t in range(NT):
                    ps = psum_pool.tile([128, 512], f32)
                    nc.tensor.matmul(
                        ps,
                        AAh[:, pc * 128:(pc + 1) * 128],
                        BBh[:, t * 512:(t + 1) * 512],
                        start=True,
                        stop=False,
                    )
                    last_mm = nc.tensor.matmul(
                        ps,
                        AAl[:, pc * 128:(pc + 1) * 128],
                        BBl[:, t * 512:(t + 1) * 512],
                        start=False,
                        stop=True,
                    )
                    nc.scalar.activation(
                        C[t][:, pc * 512:(pc + 1) * 512], ps,
                        mybir.ActivationFunctionType.Relu,
                    )

            tt_eng = nc.vector if TT_ENGINE == "vector" else nc.gpsimd

            # DTW scans, tiles row-interleaved; output shipped per row-quad.
            odmas, last_ofence, last_scan = [], None, [None] * NT
            for p in range(P):
                for t in range(NT):
                    base, prev = p * W, (p - 1) * W
                    if p % 4 == 0:
                        pc = p // 4
                        nc.vector.tensor_copy(
                            factd[t * NCH + pc],
                            C[t][0:1, pc * 512:pc * 512 + 1],
                        )
                    if p == 0:
                        last_scan[t] = nc.vector.tensor_tensor_scan(
                            Dt[t][:, 0:W], bigm, C[t][:, 0:W], 0.0,
                            mybir.AluOpType.min, mybir.AluOpType.add,
                        )
                    else:
                        tt_eng.tensor_tensor(
                            mts[t][:, 1:W],
                            Dt[t][:, prev:prev + W - 1],
                            Dt[t][:, prev + 1:prev + W],
                            mybir.AluOpType.min,
                        )
                        last_scan[t] = nc.vector.tensor_tensor_scan(
                            Dt[t][:, base:base + W], mts[t],
                            C[t][:, base:base + W], Dt[t][:, prev:prev + 1],
                            mybir.AluOpType.min, mybir.AluOpType.add,
                        )
                if p % 4 == 3:
                    q = p // 4
                    for t in range(NT):
                        dv = Dt[t].rearrange("q (p w) -> q p w", p=P)
                        nc.scalar.activation(
                            ots[t][:, 4 * q:4 * q + 4, :],
                            dv[:, 4 * q:4 * q + 4, W - WO:W],
                            mybir.ActivationFunctionType.Sqrt,
                            scale=1.0 / (P * P),
                        )
                        # ACT wait-absorber: the DMA trigger has no sync
                        # wait slot, so a 1-elem copy soaks the sqrt wait
                        # and the trigger follows in ACT queue order.
                        ofence = nc.scalar.activation(
                            facta, ots[t][0:1, 4 * q + 3, WO - 1:WO],
                            mybir.ActivationFunctionType.Copy,
                        )
                        last_ofence = ofence
                        odma = nc.scalar.dma_start(
                            out=out_d[t * 4:(t + 1) * 4, :, 4 * q:4 * q + 4, :],
                            in_=ots[t][:, 4 * q:4 * q + 4, :],
                        )
                        add_dep_helper(
                            odma.ins, ofence.ins, sync=False,
                            reason="out DMA after ACT wait-absorber",
                        )
                        odmas.append(odma)

            # Tail: feed every proc's final tick into the sync sequencer so
            # the kernel-tail drain's single-slot wait list elides.
            tail_deps = odmas + [last_ofence, last_mm] + last_scan
            prev_nop = None
            for td in tail_deps:
                nop = nc.sync.nop()
                add_dep_helper(
                    nop.ins, td.ins, sync=True,
                    reason="drain pre-absorber: sync waits on proc tail",
                )
                if prev_nop is not None:
                    add_dep_helper(
                        nop.ins, prev_nop.ins, sync=False,
                        reason="keep nop chain ordered",
                    )
                prev_nop = nop
    return nc


def make_in_maps(x: np.ndarray, patts: np.ndarray) -> list[dict[str, np.ndarray]]:
    x = np.ascontiguousarray(x, dtype=np.float32)
    patts = np.ascontiguousarray(patts, dtype=np.float32)

    # L[d', k, p]: -2*patts for d'<8, pnorm at d'=8, ones at d'=9.
    pnorm = (patts * patts).sum(axis=1)                          # (k, p)
    L = np.empty((DAUG, K, P), np.float32)
    L[:D] = -2.0 * patts.transpose(1, 0, 2)
    L[D] = pnorm
    L[D + 1] = 1.0

    # AA[(d', n', p'), (pc, nn, k)] = delta(nn==n') * L[d', k, 4*pc+p']
    Lr = L.reshape(DAUG, K, NCH, 4)                              # [d', k, pc, p']
    AA = np.zeros((DAUG, 4, 4, NCH, 4, K), np.float32)           # [d',n',p',pc,nn,k]
    for nn in range(4):
        AA[:, nn, :, :, nn, :] = Lr.transpose(0, 3, 2, 1)        # [d', p', pc, k]
    AA = AA.reshape(DAUG * 16, NCH * 4 * K)
    AA_hi = AA[:D * 16].astype(ml_dtypes.bfloat16)               # [128, 1024]
    AA_lo = AA[D * 16:].astype(ml_dtypes.bfloat16)               # [32, 1024]

    in_maps = []
    for c in range(NCORES):
        xs = x[c * NLOC:(c + 1) * NLOC]                          # (8, 8, 128)
        xnorm = (xs * xs).sum(axis=1)                            # (8, 128)
        # R[d', n_loc, w]: x for d'<8, ones at d'=8, xnorm at d'=9.
        R = np.empty((DAUG, NLOC, W), np.float32)
        R[:D] = xs.transpose(1, 0, 2)
        R[D] = 1.0
        R[D + 1] = xnorm
        # BB[(d', n', p'), (t, i, w)] = delta(i==p') * R[d', 4t+n', w]
        BB = np.zeros((DAUG, 4, 4, NT, 4, W), np.float32)        # [d',n',p',t,i,w]
        for i in range(4):
            BB[:, :, i, :, i, :] = R.reshape(DAUG, NT, 4, W).transpose(0, 2, 1, 3)
        BB = BB.reshape(DAUG * 16, NT * 4 * W)
        BB_hi = BB[:D * 16].astype(ml_dtypes.bfloat16)           # [128, 1024]
        BB_lo = BB[D * 16:].astype(ml_dtypes.bfloat16)           # [32, 1024]
        in_maps.append({
            "hi": np.concatenate([AA_hi, BB_hi], axis=1),
            "lo": np.concatenate([AA_lo, BB_lo], axis=1),
        })
    return in_maps


_program_cache: bass.Bass | None = None


def kernel(x: np.ndarray, patts: np.ndarray) -> np.ndarray:
    global _program_cache
    if _program_cache is None:
        _program_cache = build_program()
    nc = _program_cache
    in_maps = make_in_maps(x, patts)
    res = run_bass_kernel_spmd(nc, in_maps, list(range(NCORES)))
    return np.concatenate([r["out"] for r in res.results], axis=0)


if __name__ == "__main__":
    rng = np.random.default_rng(0)
    x = rng.standard_normal((N, D, W), dtype=np.float32)
    patts = rng.standard_normal((K, D, P), dtype=np.float32)
    out = kernel(x, patts)
    print(out.shape, out.dtype)


# revision 35
# speedup vs baseline: 1.0241x; 1.0241x over previous
"""DTW frames layer on 8 Trainium2 NeuronCores.

Reference computation (per (n, k) problem):
    cost[p, w] = max(0, ||x[n, :, w] - patts[k, :, p]||^2)          (P=32, W=128)
    dtw[0, w]  = cumsum_w cost[0, w]
    dtw[p, 0]  = cumsum_p cost[p, 0]
    dtw[p, w]  = cost[p, w] + min(dtw[p, w-1], dtw[p-1, w-1], dtw[p-1, w])
    out        = sqrt(dtw[:, -32:]) / 32

Strategy (v3):
  - Data-parallel over batch n: each of the 8 cores owns n_loc = 8 rows of x,
    patterns replicated. Per core, two problem tiles of 128 partitions each
    (4 n x 32 k).
  - The cost matrix is produced DIRECTLY in scan layout (partitions =
    (nn, k), free = (p, w)) by delta-masked bf16 matmuls, eliminating any
    SBUF->SBUF partition regroup:
      PSUM[(nn,k), (i,w)] = sum_{d',n',p'} AA[(d',n',p'), (nn,k)]
                                         * BB[(d',n',p'), (i,w)]
    with AA = delta(nn==n') * L[d',k,4pc+p'] and BB = delta(i==p') *
    R[d',4t+n',w], where L/R carry [-2*patts | pnorm | 1] and
    [x | 1 | xnorm]. Contraction = (d' in 10, n' in 4, p' in 4) = 160 rows,
    split into a 128-row and a 32-row matmul accumulating into one PSUM
    bank. Operands are host-prepared (masking is free on the CPU); bf16
    keeps the PE ~5x faster than fp32 and costs only ~8e-4 relative error.
  - Input is shipped as 4 DMAs over both HWDGE rings, ordered so the first
    chunk's operands land first; PSUM pool bufs=2 pins the first eviction's
    semaphore wait to its true producer (with deep PSUM rotation the Tile
    scheduler assigns it a much later PE tick, costing ~3 us of ramp).
  - ReLU on eviction (ACT) drops each PSUM bank into C[t] at the final
    (p, w) column layout - pure dense 2D ops, no data movement.
  - DTW row recurrence on the DVE tensor_tensor_scan instruction:
    state = (m[t] min state) add c[t], one instruction per table row, where
    m[t] = min(dtw[p-1, t-1], dtw[p-1, t]) is one shifted tensor_tensor min
    (also DVE: the Pool engine's stock tensor_tensor has no min ucode).
    Column 0 of the m operand stays at +BIG so element 0 of each scan is
    initial + cost (the first-column cumsum), initial = prev row's column 0.
    The two tiles' chains are emitted row-interleaved so the in-order DVE
    fills one chain's dependency-turnaround gap with the other chain's op.
    The scan ISA op has no sync-wait slot, so a tiny same-engine copy
    absorbs each quad's cross-engine evict semaphore.
  - Output is pipelined per row-quad: after rows 4q..4q+3 of a tile are
    scanned, one Sqrt activation (scale=1/1024 => sqrt(dtw)/32) and one
    contiguous 64 KB DMA ship them to HBM, so only the last quad's store
    lands in the kernel tail. DMA triggers can't hold waits either; a
    1-elem ACT copy fences each trigger in queue order.
"""

import numpy as np
import ml_dtypes

import concourse.bass as bass
import concourse.mybir as mybir
import concourse.tile as tile
from concourse.bass_utils import run_bass_kernel_spmd

N, D, W = 64, 8, 128      # x: (N, D, W)
K, P = 32, 32             # patts: (K, D, P)
WO = 32                   # output keeps last WO columns of the DTW table
NCORES = 8
NLOC = N // NCORES        # 8 batch rows per core
NT = 2                    # problem tiles per core: (4 n x 32 k) = 128 partitions
NCH = P // 4              # p-quad chunks per tile
DAUG = D + 2              # augmented d' rows: [-2*patts | pnorm | 1]
BIG = 1e30

f32 = mybir.dt.float32
bf16 = mybir.dt.bfloat16


def build_program() -> bass.Bass:
    from concourse.tile import add_dep_helper

    nc = bass.Bass()
    # Input split for an early first matmul: sync ring carries what chunk 0
    # tile 0 needs plus tile 1's moving operand; scalar ring carries the
    # norm rows and the remaining stationaries.
    hia_d = nc.dram_tensor("hia", (128, 768), bf16, kind="ExternalInput")
    hib_d = nc.dram_tensor("hib", (128, 512), bf16, kind="ExternalInput")
    lo_d = nc.dram_tensor("lo", (32, 2048), bf16, kind="ExternalInput")
    hic_d = nc.dram_tensor("hic", (128, 768), bf16, kind="ExternalInput")
    out_d = nc.dram_tensor("out", (NLOC, K, P, WO), f32, kind="ExternalOutput")

    with tile.TileContext(nc) as tc:
        with (
            tc.tile_pool(name="consts", bufs=1) as consts,
            tc.tile_pool(name="psum", bufs=2, space="PSUM") as psum_pool,
            tc.tile_pool(name="cbuf", bufs=1) as c_pool,
            tc.tile_pool(name="dbuf", bufs=1) as d_pool,
            tc.tile_pool(name="mbuf", bufs=1) as m_pool,
            tc.tile_pool(name="obuf", bufs=1) as o_pool,
        ):
            AAh = consts.tile([128, NCH * 128], bf16, tag="AAh", name="AAh")
            BBh = consts.tile([128, NT * 512], bf16, tag="BBh", name="BBh")
            lo_s = consts.tile([32, 2048], bf16, tag="lo_s", name="lo_s")
            nc.sync.dma_start(out=AAh[:, 0:256], in_=hia_d[:, 0:256])
            nc.sync.dma_start(out=BBh[:, 0:512], in_=hia_d[:, 256:768])
            nc.sync.dma_start(out=BBh[:, 512:1024], in_=hib_d[:, :])
            nc.scalar.dma_start(out=lo_s, in_=lo_d[:, :])
            nc.scalar.dma_start(out=AAh[:, 256:1024], in_=hic_d[:, :])
            AAl = lo_s[:, 0:NCH * 128]
            BBl = lo_s[:, NCH * 128:NCH * 128 + NT * 512]

            C = [
                c_pool.tile([128, P * W], f32, tag=f"C{t}", name=f"C{t}")
                for t in range(NT)
            ]
            Dt = [
                d_pool.tile([128, P * W], f32, tag=f"D{t}", name=f"D{t}")
                for t in range(NT)
            ]
            mts = [
                m_pool.tile([128, W], f32, tag=f"mt{t}", name=f"mt{t}")
                for t in range(NT)
            ]
            bigm = m_pool.tile([128, W], f32, tag="bigm", name="bigm")
            ots = [
                o_pool.tile([128, P, WO], f32, tag=f"ot{t}", name=f"ot{t}")
                for t in range(NT)
            ]
            # DVE wait absorbers: the scan ISA op (S2S2D2_STT) has no sync
            # wait slot, so a tiny same-engine tensor_copy soaks each
            # quad's cross-engine evict semaphore ahead of its first scan.
            factd = [
                consts.tile([1, 1], f32, name=f"factd{i}", tag=f"factd{i}")
                for i in range(NT * NCH)
            ]
            facta = consts.tile([1, 1], f32, name="facta", tag="facta")
            nc.vector.memset(bigm, BIG)
            for t in range(NT):
                nc.vector.memset(mts[t], BIG)

            # Cost production: one PSUM bank per (pc, t) in scan layout.
            last_mm = None
            for pc in range(NCH):
                for t in range(NT):
                    ps = psum_pool.tile([128, 512], f32)
                    nc.tensor.matmul(
                        ps,
                        AAh[:, pc * 128:(pc + 1) * 128],
                        BBh[:, t * 512:(t + 1) * 512],
                        start=True,
                        stop=False,
                    )
                    last_mm = nc.tensor.matmul(
                        ps,
                        AAl[:, pc * 128:(pc + 1) * 128],
                        BBl[:, t * 512:(t + 1) * 512],
                        start=False,
                        stop=True,
                    )
                    nc.scalar.activation(
                        C[t][:, pc * 512:(pc + 1) * 512], ps,
                        mybir.ActivationFunctionType.Relu,
                    )

            # DTW scans, tiles row-interleaved; output shipped per row-quad.
            odmas, last_ofence, last_scan = [], None, [None] * NT
            for p in range(P):
                for t in range(NT):
                    base, prev = p * W, (p - 1) * W
                    if p % 4 == 0:
                        pc = p // 4
                        nc.vector.tensor_copy(
                            factd[t * NCH + pc],
                            C[t][0:1, pc * 512:pc * 512 + 1],
                        )
                    if p == 0:
                        last_scan[t] = nc.vector.tensor_tensor_scan(
                            Dt[t][:, 0:W], bigm, C[t][:, 0:W], 0.0,
                            mybir.AluOpType.min, mybir.AluOpType.add,
                        )
                    else:
                        nc.vector.tensor_tensor(
                            mts[t][:, 1:W],
                            Dt[t][:, prev:prev + W - 1],
                            Dt[t][:, prev + 1:prev + W],
                            mybir.AluOpType.min,
                        )
                        last_scan[t] = nc.vector.tensor_tensor_scan(
                            Dt[t][:, base:base + W], mts[t],
                            C[t][:, base:base + W], Dt[t][:, prev:prev + 1],
                            mybir.AluOpType.min, mybir.AluOpType.add,
                        )
                if p % 4 == 3:
                    q = p // 4
                    for t in range(NT):
                        dv = Dt[t].rearrange("q (p w) -> q p w", p=P)
                        nc.scalar.activation(
                            ots[t][:, 4 * q:4 * q + 4, :],
                            dv[:, 4 * q:4 * q + 4, W - WO:W],
                            mybir.ActivationFunctionType.Sqrt,
                            scale=1.0 / (P * P),
                        )
                        ofence = nc.scalar.activation(
                            facta, ots[t][0:1, 4 * q + 3, WO - 1:WO],
                            mybir.ActivationFunctionType.Copy,
                        )
                        last_ofence = ofence
                        odma = nc.scalar.dma_start(
                            out=out_d[t * 4:(t + 1) * 4, :, 4 * q:4 * q + 4, :],
                            in_=ots[t][:, 4 * q:4 * q + 4, :],
                        )
                        add_dep_helper(
                            odma.ins, ofence.ins, sync=False,
                            reason="out DMA after ACT wait-absorber",
                        )
                        odmas.append(odma)

            # Tail: feed every proc's final tick into the sync sequencer so
            # the kernel-tail drain's single-slot wait list elides.
            tail_deps = odmas + [last_ofence, last_mm] + last_scan
            prev_nop = None
            for td in tail_deps:
                nop = nc.sync.nop()
                add_dep_helper(
                    nop.ins, td.ins, sync=True,
                    reason="drain pre-absorber: sync waits on proc tail",
                )
                if prev_nop is not None:
                    add_dep_helper(
                        nop.ins, prev_nop.ins, sync=False,
                        reason="keep nop chain ordered",
                    )
                prev_nop = nop
    return nc


def make_in_maps(x: np.ndarray, patts: np.ndarray) -> list[dict[str, np.ndarray]]:
    x = np.ascontiguousarray(x, dtype=np.float32)
    patts = np.ascontiguousarray(patts, dtype=np.float32)

    # L[d', k, p]: -2*patts for d'<8, pnorm at d'=8, ones at d'=9.
    pnorm = (patts * patts).sum(axis=1)                          # (k, p)
    L = np.empty((DAUG, K, P), np.float32)
    L[:D] = -2.0 * patts.transpose(1, 0, 2)
    L[D] = pnorm
    L[D + 1] = 1.0

    # AA[(d', n', p'), (pc, nn, k)] = delta(nn==n') * L[d', k, 4*pc+p']
    Lr = L.reshape(DAUG, K, NCH, 4)                              # [d', k, pc, p']
    AA = np.zeros((DAUG, 4, 4, NCH, 4, K), np.float32)           # [d',n',p',pc,nn,k]
    for nn in range(4):
        AA[:, nn, :, :, nn, :] = Lr.transpose(0, 3, 2, 1)        # [d', p', pc, k]
    AA = AA.reshape(DAUG * 16, NCH * 4 * K)
    AA_hi = AA[:D * 16].astype(ml_dtypes.bfloat16)               # [128, 1024]
    AA_lo = AA[D * 16:].astype(ml_dtypes.bfloat16)               # [32, 1024]

    in_maps = []
    for c in range(NCORES):
        xs = x[c * NLOC:(c + 1) * NLOC]                          # (8, 8, 128)
        xnorm = (xs * xs).sum(axis=1)                            # (8, 128)
        # R[d', n_loc, w]: x for d'<8, ones at d'=8, xnorm at d'=9.
        R = np.empty((DAUG, NLOC, W), np.float32)
        R[:D] = xs.transpose(1, 0, 2)
        R[D] = 1.0
        R[D + 1] = xnorm
        # BB[(d', n', p'), (t, i, w)] = delta(i==p') * R[d', 4t+n', w]
        BB = np.zeros((DAUG, 4, 4, NT, 4, W), np.float32)        # [d',n',p',t,i,w]
        for i in range(4):
            BB[:, :, i, :, i, :] = R.reshape(DAUG, NT, 4, W).transpose(0, 2, 1, 3)
        BB = BB.reshape(DAUG * 16, NT * 4 * W)
        BB_hi = BB[:D * 16].astype(ml_dtypes.bfloat16)           # [128, 1024]
        BB_lo = BB[D * 16:].astype(ml_dtypes.bfloat16)           # [32, 1024]
        in_maps.append({
            "hia": np.concatenate([AA_hi[:, 0:256], BB_hi[:, 0:512]], axis=1),
            "hib": np.ascontiguousarray(BB_hi[:, 512:1024]),
            "lo": np.concatenate([AA_lo, BB_lo], axis=1),
            "hic": np.ascontiguousarray(AA_hi[:, 256:1024]),
        })
    return in_maps


_program_cache: bass.Bass | None = None


def kernel(x: np.ndarray, patts: np.ndarray) -> np.ndarray:
    global _program_cache
    if _program_cache is None:
        _program_cache = build_program()
    nc = _program_cache
    in_maps = make_in_maps(x, patts)
    res = run_bass_kernel_spmd(nc, in_maps, list(range(NCORES)))
    return np.concatenate([r["out"] for r in res.results], axis=0)


if __name__ == "__main__":
    rng = np.random.default_rng(0)
    x = rng.standard_normal((N, D, W), dtype=np.float32)
    patts = rng.standard_normal((K, D, P), dtype=np.float32)
    out = kernel(x, patts)
    print(out.shape, out.dtype)
